# revision 63
# baseline (speedup 1.0000x reference)
"""CRF forward-algorithm (log partition) kernel for 8 Trainium2 NeuronCores.

Strategy: fp8 DoubleRow overlapped-segment exp-space scan.

The reference recurrence  fv' = logsumexp_prev(fv + T) + feat  is, in exp
space, a linear matvec chain  v' = diag(e_t) @ M @ v  with M = exp(T) fixed.
We split the T=16384 steps into S=4096 segments of L=4 and run all segments
in parallel from a guess vector, 512 columns per core, as dense
[128x(2x128)] x [128x(2x512)] fp8 DoubleRow matmuls (256-wide contraction,
2 MACs/cell/cycle — 2x bf16 PE throughput).  Products of positive matrices
contract exponentially toward rank-1 (Perron-Frobenius), so each column
only needs a short warmup to forget its guess: column j starts D=1 steps
early (at absolute step j*L-D) and the scalar mismatch at each segment
junction (kappa) is read off on the host by comparing column j's endpoint
with column j+1's warmed-up snapshot at the same absolute time.  No second
phase, no collectives.  alpha = lse(final column) + sum(kappa) + folded
scale bookkeeping, assembled on host in fp64.

Magnitudes are kept inside fp8 e4m3 range by a constant per-step scale
e^-sigma (sigma = log lambda1(M) + 1/2, host-estimated by power iteration).
Per step, blocks 2-7 apply it on the scalar engine (psum*alpha -> bf16)
followed by a DVE multiply with fp8 emissions; blocks 0-1 (and the whole
last step) instead multiply psum directly with bf16 pre-scaled emissions on
the DVE — balancing the two engines so neither chain ever stalls the PE.
Matmuls are emitted pair-major, there are ~4us of warm-up matmuls to open
the HAM clock gate during the prologue DMAs, and the DoubleRow stream then
runs gapless at ~216 ns per 128x256x512 matmul.
"""

import numpy as np
import ml_dtypes

import concourse.bass as bass
import concourse.bacc as bacc
import concourse.mybir as mybir
import concourse.tile as tile

FP8_NP = ml_dtypes.float8_e4m3   # TRN float8e4: max 240
FP8 = mybir.dt.float8e4
BF16 = mybir.dt.bfloat16
F32 = mybir.dt.float32

SEQ_LEN = 16384
N_TAGS = 1024
START_IDX = 1022
STOP_IDX = 1023
NB = 8                  # 1024 tags = 8 planes of 128 partitions
L = 4                   # segment length (steps)
D = 1                   # guess-warmup depth (steps)
NSTEP = L + D
S = SEQ_LEN // L        # 4096 segments = columns
NCORES = 8
FD = S // NCORES        # 512 columns per core
GUESS = 1.0
BETA = 224.0            # col-0 one-hot init value
NDIR = 2                # blocks 0..NDIR-1 use the direct-DVE path

_CACHE = {}


def _build_program():
    nc = bacc.Bacc("TRN2", target_bir_lowering=False, debug=False)
    mt = nc.dram_tensor("mt", [128, NB, N_TAGS], FP8, kind="ExternalInput")
    vinit = nc.dram_tensor("vinit", [128, NB, FD], FP8, kind="ExternalInput")
    ems = nc.dram_tensor("ems", [NSTEP, 128, NB, FD], FP8, kind="ExternalInput")
    # planes 0-1 of the emissions, pre-scaled by e^-sigma, in bf16: lets
    # blocks 0-1 skip the scalar engine (single DVE op psum*em -> fp8)
    ems2 = nc.dram_tensor("ems2", [NSTEP, 128, NDIR, FD], BF16, kind="ExternalInput")
    # last step: ALL planes pre-scaled bf16 -> pure-DVE tail (no ACT chain)
    emsl = nc.dram_tensor("emsl", [128, NB, FD], BF16, kind="ExternalInput")
    alph = nc.dram_tensor("alph", [128, 1], F32, kind="ExternalInput")
    snapd = nc.dram_tensor("snapd", [128, NB, FD], FP8, kind="ExternalOutput")
    snapl = nc.dram_tensor("snapl", [128, NB, FD], FP8, kind="ExternalOutput")
    vfin = nc.dram_tensor("vfin", [128, NB, FD], FP8, kind="ExternalOutput")

    with tile.TileContext(nc) as tc:
        with (
            tc.tile_pool(name="mpool", bufs=1) as mpool,
            tc.tile_pool(name="vpool", bufs=2) as vpool,
            tc.tile_pool(name="epool", bufs=NSTEP) as epool,
            tc.tile_pool(name="tpool", bufs=4) as tpool,
            tc.tile_pool(name="pspool", bufs=1, space="PSUM") as pspool,
        ):
            # chunked loads: 4 parallel DMA queues so the stationary matrix
            # lands in ~1/4 the single-queue time
            mt_sb = mpool.tile([128, NB, N_TAGS], FP8)
            v = vpool.tile([128, NB, FD], FP8, tag="v")
            for g in range(4):
                pl = slice(2 * g, 2 * g + 2)
                nc.sync.dma_start(mt_sb[:, pl, :], mt[:, pl, :])
                nc.sync.dma_start(v[:, pl, :], vinit[:, pl, :])
            al_sb = mpool.tile([128, 1], F32)
            nc.sync.dma_start(al_sb[:], alph[:])
            # dummy activation: trigger the one-time ACT table load early so
            # it does not gate the first step's scalar-engine chain
            warm = mpool.tile([128, 1], BF16)
            nc.scalar.activation(warm[:], al_sb[:],
                                 mybir.ActivationFunctionType.Copy, scale=1.0)
            # PE warm-up: ~4us of dummy matmuls spanning the prologue DMAs so
            # the HAM clock gate opens before the real stream starts
            wsrc = mpool.tile([128, 256], FP8)
            nc.vector.memset(wsrc[:], 1.0)
            ps0w = pspool.tile([128, FD], F32, tag="ps0")
            for _ in range(30):
                nc.tensor.matmul(ps0w[0:128, 0:256], wsrc[:, 0:128], wsrc[:],
                                 start=True, stop=True, skip_group_check=True)

            for s in range(NSTEP):
                last = s == NSTEP - 1
                if last:
                    etl = epool.tile([128, NB, FD], BF16, tag="eml")
                    nc.sync.dma_start(etl[:], emsl[:])
                else:
                    et = epool.tile([128, NB, FD], FP8, tag="em")
                    nc.sync.dma_start(et[:], ems[s])
                    et2 = epool.tile([128, NDIR, FD], BF16, tag="em2")
                    nc.sync.dma_start(et2[:], ems2[s])
                nv = vpool.tile([128, NB, FD], FP8, tag="v")
                pss = []
                for mb in range(NB):
                    ps = pspool.tile([128, FD], F32, tag=f"ps{mb}")
                    pss.append(ps)
                # paired emission order: banks (2p, 2p+1) run g0-g2 then both
                # g3s, so each pair's accumulation completes ~1.7us apart and
                # the ACT->DVE chain for early planes finishes mid-step —
                # the next step's matmuls never wait on plane or psum deps.
                for p in range(4):
                    order = [(2 * p + b, g) for b in range(2) for g in range(3)]
                    order += [(2 * p, 3), (2 * p + 1, 3)]
                    for mb, g in order:
                        nc.tensor.matmul(
                            pss[mb][:],
                            mt_sb[:, 2 * g:2 * g + 2, mb * 128:(mb + 1) * 128],
                            v[:, 2 * g:2 * g + 2, :],
                            start=(g == 0),
                            stop=(g == 3),
                            perf_mode=mybir.MatmulPerfMode.DoubleRow,
                            skip_group_check=True,
                        )
                if last:
                    # pure-DVE tail; dump vfin in plane-pair chunks as ready
                    for mb in range(NB):
                        nc.vector.tensor_mul(nv[:, mb, :], pss[mb][:],
                                             etl[:, mb, :])
                        if mb % 2 == 1:
                            pl = slice(mb - 1, mb + 1)
                            nc.sync.dma_start(vfin[:, pl, :], nv[:, pl, :])
                else:
                    for mb in range(NDIR):
                        nc.vector.tensor_mul(nv[:, mb, :], pss[mb][:],
                                             et2[:, mb, :])
                    for mb in range(NDIR, NB):
                        tmp = tpool.tile([128, FD], BF16, tag="tmp")
                        nc.scalar.activation(
                            tmp[:], pss[mb][:],
                            mybir.ActivationFunctionType.Copy,
                            scale=al_sb[:],
                        )
                        nc.vector.tensor_mul(nv[:, mb, :], tmp[:], et[:, mb, :])
                    out_d = {D: snapd, L: snapl}.get(s + 1)
                    if out_d is not None:
                        nc.sync.dma_start(out_d[:], nv[:])
                v = nv

    nc.compile()
    return nc


def _sigma(transitions):
    """log of the dominant per-step growth: log lambda1(exp(T)) + E[log-normal
    mean of emissions] (=1/2 for N(0,1) decoded)."""
    M = np.exp(transitions.astype(np.float64))
    x = np.ones(N_TAGS)
    lam = 1.0
    for _ in range(20):
        x = M @ x
        lam = x.max()
        x /= lam
    return float(np.log(lam) + 0.5)


def _prepare_core_inputs(decoded, transitions):
    Mt = np.exp(transitions.astype(np.float32)).T        # [prev, next]
    mt = np.ascontiguousarray(
        Mt.reshape(NB, 128, N_TAGS).transpose(1, 0, 2)
    ).astype(FP8_NP)                                     # [p, kb, next]

    sigma = _sigma(transitions)
    E8 = np.exp(decoded.astype(np.float32)).astype(FP8_NP)   # [T, N]
    # pre-scaled bf16 copy of the first NDIR tag-planes (for direct-DVE)
    E16 = np.exp(decoded[:, :NDIR * 128].astype(np.float32)
                 - np.float32(sigma)).astype(ml_dtypes.bfloat16)

    # column schedule: col j>=1 local s -> t = j*L-D+s ; col 0 -> t = s
    t_of = np.empty((NSTEP, S), dtype=np.int64)
    steps = np.arange(NSTEP)
    t_of[:, 0] = steps
    t_of[:, 1:] = (np.arange(1, S) * L - D)[None, :] + steps[:, None]

    A = E8[t_of]                                         # [NSTEP, S, N]
    A = A.reshape(NSTEP, S, NB, 128).transpose(0, 3, 2, 1)   # [NSTEP, p, kb, S]
    A2 = E16[t_of]                                       # [NSTEP, S, NDIR*128]
    A2 = A2.reshape(NSTEP, S, NDIR, 128).transpose(0, 3, 2, 1)
    # full-width pre-scaled bf16 emissions for the last step
    Al = np.exp(decoded[t_of[NSTEP - 1]].astype(np.float32)
                - np.float32(sigma)).astype(ml_dtypes.bfloat16)  # [S, N]
    Al = Al.reshape(S, NB, 128).transpose(2, 1, 0)       # [p, kb, S]

    vbase = np.full((128, NB, FD), GUESS, dtype=FP8_NP)

    alph = np.full((128, 1), np.exp(-sigma), dtype=np.float32)
    in_maps = []
    for c in range(NCORES):
        ems = np.ascontiguousarray(A[:, :, :, c * FD:(c + 1) * FD])
        ems2 = np.ascontiguousarray(A2[:, :, :, c * FD:(c + 1) * FD])
        vin = vbase.copy()
        if c == 0:
            vin[:, :, 0] = FP8_NP(0.0)
            vin[START_IDX % 128, START_IDX // 128, 0] = FP8_NP(BETA)
        in_maps.append({"mt": mt, "vinit": vin, "ems": ems, "ems2": ems2,
                        "emsl": np.ascontiguousarray(Al[:, :, c * FD:(c + 1) * FD]),
                        "alph": alph})
    return in_maps, sigma


def _assemble(transitions, results, sigma, decoded):
    """Host-side kappa extraction + terminal logsumexp (tiny, fp64)."""
    def cat(key):
        # [128, NB, FD] per core -> [N_TAGS, S]
        return np.concatenate(
            [results[c][key].astype(np.float64).transpose(1, 0, 2).reshape(N_TAGS, FD)
             for c in range(NCORES)], axis=1)

    u = cat("snapd")                 # state at time j*L      (warmed guess)
    w = cat("vfin")                  # state at time (j+1)*L  (endpoint)
    z = results[0]["snapl"].astype(np.float64).transpose(1, 0, 2).reshape(N_TAGS, FD)[:, 0]

    num = np.concatenate([z[:, None], w[:, 1:S - 1]], axis=1)   # [N, S-1]
    den = u[:, 1:]
    valid = (num > 0) & (den > 0) & np.isfinite(num)
    with np.errstate(divide="ignore", invalid="ignore"):
        dlt = np.where(valid, np.log(num) - np.log(den), np.nan)
    kap = np.nanmedian(dlt, axis=0)                              # [S-1]

    with np.errstate(divide="ignore"):
        logx = np.log(w[:, S - 1]) + kap.sum() + SEQ_LEN * sigma - np.log(BETA)
    term = logx + transitions[STOP_IDX].astype(np.float64)
    term = term[np.isfinite(term)]
    mx = term.max()
    alpha = mx + np.log(np.exp(term - mx).sum())
    return alpha


def kernel(decoded, transitions, raw_outputs=None, outputs=None, _backend="hw"):
    decoded = np.asarray(decoded, dtype=np.float32)
    transitions = np.asarray(transitions, dtype=np.float32)

    in_maps, sigma = _prepare_core_inputs(decoded, transitions)

    if "nc" not in _CACHE:
        _CACHE["nc"] = _build_program()
    nc = _CACHE["nc"]

    if _backend == "sim":
        from concourse.bass_interp import CoreSim
        results = []
        for c in range(NCORES):
            sim = CoreSim(nc, trace=False)
            for k, v in in_maps[c].items():
                sim.tensor(k)[:] = v
            sim.simulate()
            results.append({k: np.array(sim.tensor(k))
                            for k in ("snapd", "snapl", "vfin")})
    else:
        from concourse.bass_utils import run_bass_kernel_spmd
        res = run_bass_kernel_spmd(nc, in_maps, list(range(NCORES)))
        results = res.results

    alpha = _assemble(transitions, results, sigma, decoded)
    return np.float32(alpha)


# revision 66
# speedup vs baseline: 1.0640x; 1.0640x over previous
"""CRF forward-algorithm (log partition) kernel for 8 Trainium2 NeuronCores.

Strategy: fp8 DoubleRow overlapped-segment exp-space scan.

The reference recurrence  fv' = logsumexp_prev(fv + T) + feat  is, in exp
space, a linear matvec chain  v' = diag(e_t) @ M @ v  with M = exp(T) fixed.
We split the T=16384 steps into S=4096 segments of L=4 and run all segments
in parallel from a guess vector, 512 columns per core, as dense
[128x(2x128)] x [128x(2x512)] fp8 DoubleRow matmuls (256-wide contraction,
2 MACs/cell/cycle — 2x bf16 PE throughput).  Products of positive matrices
contract exponentially toward rank-1 (Perron-Frobenius), so each column
only needs a short warmup to forget its guess: column j starts D=1 steps
early (at absolute step j*L-D) and the scalar mismatch at each segment
junction (kappa) is read off on the host by comparing column j's endpoint
with column j+1's warmed-up snapshot at the same absolute time.  No second
phase, no collectives.  alpha = lse(final column) + sum(kappa) + folded
scale bookkeeping, assembled on host in fp64.

Magnitudes are kept inside fp8 e4m3 range by a constant per-step scale
e^-sigma (sigma = log lambda1(M) + 1/2, host-estimated by power iteration).
Per step, blocks 2-7 apply it on the scalar engine (psum*alpha -> bf16)
followed by a DVE multiply with fp8 emissions; blocks 0-1 (and the whole
last step) instead multiply psum directly with bf16 pre-scaled emissions on
the DVE — balancing the two engines so neither chain ever stalls the PE.
Matmuls are emitted pair-major, there are ~4us of warm-up matmuls to open
the HAM clock gate during the prologue DMAs, and the DoubleRow stream then
runs gapless at ~216 ns per 128x256x512 matmul.
"""

import numpy as np
import ml_dtypes

import concourse.bass as bass
import concourse.bacc as bacc
import concourse.mybir as mybir
import concourse.tile as tile

FP8_NP = ml_dtypes.float8_e4m3   # TRN float8e4: max 240
FP8 = mybir.dt.float8e4
BF16 = mybir.dt.bfloat16
F32 = mybir.dt.float32

SEQ_LEN = 16384
N_TAGS = 1024
START_IDX = 1022
STOP_IDX = 1023
NB = 8                  # 1024 tags = 8 planes of 128 partitions
L = 4                   # segment length (steps)
D = 1                   # guess-warmup depth (steps)
NSTEP = L + D
S = SEQ_LEN // L        # 4096 segments = columns
NCORES = 8
FD = S // NCORES        # 512 columns per core
GUESS = 1.0
BETA = 224.0            # col-0 one-hot init value
NDIR = 2                # blocks 0..NDIR-1 use the direct-DVE path

_CACHE = {}


def _build_program():
    nc = bacc.Bacc("TRN2", target_bir_lowering=False, debug=False)
    mt = nc.dram_tensor("mt", [128, NB, N_TAGS], FP8, kind="ExternalInput")
    vinit = nc.dram_tensor("vinit", [128, NB, FD], FP8, kind="ExternalInput")
    ems = nc.dram_tensor("ems", [NSTEP, 128, NB, FD], FP8, kind="ExternalInput")
    # planes 0-1 of the emissions, pre-scaled by e^-sigma, in bf16: lets
    # blocks 0-1 skip the scalar engine (single DVE op psum*em -> fp8)
    ems2 = nc.dram_tensor("ems2", [NSTEP, 128, NDIR, FD], BF16, kind="ExternalInput")
    # last step: ALL planes pre-scaled bf16 -> pure-DVE tail (no ACT chain)
    emsl = nc.dram_tensor("emsl", [128, NB, FD], BF16, kind="ExternalInput")
    alph = nc.dram_tensor("alph", [128, 1], F32, kind="ExternalInput")
    snapd = nc.dram_tensor("snapd", [128, NB, FD], FP8, kind="ExternalOutput")
    snapl = nc.dram_tensor("snapl", [128, NB, FD], FP8, kind="ExternalOutput")
    vfin = nc.dram_tensor("vfin", [128, NB, FD], FP8, kind="ExternalOutput")

    with tile.TileContext(nc) as tc:
        with (
            tc.tile_pool(name="mpool", bufs=1) as mpool,
            tc.tile_pool(name="vpool", bufs=2) as vpool,
            tc.tile_pool(name="epool", bufs=NSTEP) as epool,
            tc.tile_pool(name="tpool", bufs=4) as tpool,
            tc.tile_pool(name="pspool", bufs=1, space="PSUM") as pspool,
        ):
            # chunked loads: 4 parallel DMA queues so the stationary matrix
            # lands in ~1/4 the single-queue time
            mt_sb = mpool.tile([128, NB, N_TAGS], FP8)
            v = vpool.tile([128, NB, FD], FP8, tag="v")
            for g in range(4):
                pl = slice(2 * g, 2 * g + 2)
                nc.sync.dma_start(mt_sb[:, pl, :], mt[:, pl, :])
                nc.sync.dma_start(v[:, pl, :], vinit[:, pl, :])
            al_sb = mpool.tile([128, 1], F32)
            nc.sync.dma_start(al_sb[:], alph[:])
            # dummy activation: trigger the one-time ACT table load early so
            # it does not gate the first step's scalar-engine chain
            warm = mpool.tile([128, 1], BF16)
            nc.scalar.activation(warm[:], al_sb[:],
                                 mybir.ActivationFunctionType.Copy, scale=1.0)
            # PE warm-up: ~4us of dummy matmuls spanning the prologue DMAs so
            # the HAM clock gate opens before the real stream starts
            wsrc = mpool.tile([128, 256], FP8)
            nc.vector.memset(wsrc[:], 1.0)
            ps0w = pspool.tile([128, FD], F32, tag="ps0")
            for _ in range(36):
                nc.tensor.matmul(ps0w[0:128, 0:256], wsrc[:, 0:128], wsrc[:],
                                 start=True, stop=True, skip_group_check=True)

            for s in range(NSTEP):
                last = s == NSTEP - 1
                if last:
                    etl = epool.tile([128, NB, FD], BF16, tag="eml")
                    nc.sync.dma_start(etl[:], emsl[:])
                else:
                    et = epool.tile([128, NB, FD], FP8, tag="em")
                    nc.sync.dma_start(et[:], ems[s])
                    et2 = epool.tile([128, NDIR, FD], BF16, tag="em2")
                    nc.sync.dma_start(et2[:], ems2[s])
                nv = vpool.tile([128, NB, FD], FP8, tag="v")
                pss = []
                for mb in range(NB):
                    ps = pspool.tile([128, FD], F32, tag=f"ps{mb}")
                    pss.append(ps)
                # paired emission order: banks (2p, 2p+1) run g0-g2 then both
                # g3s, so each pair's accumulation completes ~1.7us apart and
                # the ACT->DVE chain for early planes finishes mid-step —
                # the next step's matmuls never wait on plane or psum deps.
                for p in range(4):
                    order = [(2 * p + b, g) for b in range(2) for g in range(3)]
                    order += [(2 * p, 3), (2 * p + 1, 3)]
                    for mb, g in order:
                        nc.tensor.matmul(
                            pss[mb][:],
                            mt_sb[:, 2 * g:2 * g + 2, mb * 128:(mb + 1) * 128],
                            v[:, 2 * g:2 * g + 2, :],
                            start=(g == 0),
                            stop=(g == 3),
                            perf_mode=mybir.MatmulPerfMode.DoubleRow,
                            skip_group_check=True,
                        )
                if last:
                    # pure-DVE tail; dump vfin in plane-pair chunks as ready
                    for mb in range(NB):
                        nc.vector.tensor_mul(nv[:, mb, :], pss[mb][:],
                                             etl[:, mb, :])
                        if mb % 2 == 1:
                            pl = slice(mb - 1, mb + 1)
                            nc.sync.dma_start(vfin[:, pl, :], nv[:, pl, :])
                else:
                    for mb in range(NDIR):
                        nc.vector.tensor_mul(nv[:, mb, :], pss[mb][:],
                                             et2[:, mb, :])
                    for mb in range(NDIR, NB):
                        tmp = tpool.tile([128, FD], BF16, tag="tmp")
                        nc.scalar.activation(
                            tmp[:], pss[mb][:],
                            mybir.ActivationFunctionType.Copy,
                            scale=al_sb[:],
                        )
                        nc.vector.tensor_mul(nv[:, mb, :], tmp[:], et[:, mb, :])
                    out_d = {D: snapd, L: snapl}.get(s + 1)
                    if out_d is not None:
                        nc.sync.dma_start(out_d[:], nv[:])
                v = nv

    nc.compile()
    return nc


def _sigma(transitions):
    """log of the dominant per-step growth: log lambda1(exp(T)) + E[log-normal
    mean of emissions] (=1/2 for N(0,1) decoded)."""
    M = np.exp(transitions.astype(np.float64))
    x = np.ones(N_TAGS)
    lam = 1.0
    for _ in range(20):
        x = M @ x
        lam = x.max()
        x /= lam
    return float(np.log(lam) + 0.5)


def _prepare_core_inputs(decoded, transitions):
    Mt = np.exp(transitions.astype(np.float32)).T        # [prev, next]
    mt = np.ascontiguousarray(
        Mt.reshape(NB, 128, N_TAGS).transpose(1, 0, 2)
    ).astype(FP8_NP)                                     # [p, kb, next]

    sigma = _sigma(transitions)
    E8 = np.exp(decoded.astype(np.float32)).astype(FP8_NP)   # [T, N]
    # pre-scaled bf16 copy of the first NDIR tag-planes (for direct-DVE)
    E16 = np.exp(decoded[:, :NDIR * 128].astype(np.float32)
                 - np.float32(sigma)).astype(ml_dtypes.bfloat16)

    # column schedule: col j>=1 local s -> t = j*L-D+s ; col 0 -> t = s
    t_of = np.empty((NSTEP, S), dtype=np.int64)
    steps = np.arange(NSTEP)
    t_of[:, 0] = steps
    t_of[:, 1:] = (np.arange(1, S) * L - D)[None, :] + steps[:, None]

    A = E8[t_of]                                         # [NSTEP, S, N]
    A = A.reshape(NSTEP, S, NB, 128).transpose(0, 3, 2, 1)   # [NSTEP, p, kb, S]
    A2 = E16[t_of]                                       # [NSTEP, S, NDIR*128]
    A2 = A2.reshape(NSTEP, S, NDIR, 128).transpose(0, 3, 2, 1)
    # full-width pre-scaled bf16 emissions for the last step
    Al = np.exp(decoded[t_of[NSTEP - 1]].astype(np.float32)
                - np.float32(sigma)).astype(ml_dtypes.bfloat16)  # [S, N]
    Al = Al.reshape(S, NB, 128).transpose(2, 1, 0)       # [p, kb, S]

    vbase = np.full((128, NB, FD), GUESS, dtype=FP8_NP)

    alph = np.full((128, 1), np.exp(-sigma), dtype=np.float32)
    in_maps = []
    for c in range(NCORES):
        ems = np.ascontiguousarray(A[:, :, :, c * FD:(c + 1) * FD])
        ems2 = np.ascontiguousarray(A2[:, :, :, c * FD:(c + 1) * FD])
        vin = vbase.copy()
        if c == 0:
            vin[:, :, 0] = FP8_NP(0.0)
            vin[START_IDX % 128, START_IDX // 128, 0] = FP8_NP(BETA)
        in_maps.append({"mt": mt, "vinit": vin, "ems": ems, "ems2": ems2,
                        "emsl": np.ascontiguousarray(Al[:, :, c * FD:(c + 1) * FD]),
                        "alph": alph})
    return in_maps, sigma


def _assemble(transitions, results, sigma, decoded):
    """Host-side kappa extraction + terminal logsumexp (tiny, fp64)."""
    def cat(key):
        # [128, NB, FD] per core -> [N_TAGS, S]
        return np.concatenate(
            [results[c][key].astype(np.float64).transpose(1, 0, 2).reshape(N_TAGS, FD)
             for c in range(NCORES)], axis=1)

    u = cat("snapd")                 # state at time j*L      (warmed guess)
    w = cat("vfin")                  # state at time (j+1)*L  (endpoint)
    z = results[0]["snapl"].astype(np.float64).transpose(1, 0, 2).reshape(N_TAGS, FD)[:, 0]

    num = np.concatenate([z[:, None], w[:, 1:S - 1]], axis=1)   # [N, S-1]
    den = u[:, 1:]
    valid = (num > 0) & (den > 0) & np.isfinite(num)
    with np.errstate(divide="ignore", invalid="ignore"):
        dlt = np.where(valid, np.log(num) - np.log(den), np.nan)
    kap = np.nanmedian(dlt, axis=0)                              # [S-1]

    with np.errstate(divide="ignore"):
        logx = np.log(w[:, S - 1]) + kap.sum() + SEQ_LEN * sigma - np.log(BETA)
    term = logx + transitions[STOP_IDX].astype(np.float64)
    term = term[np.isfinite(term)]
    mx = term.max()
    alpha = mx + np.log(np.exp(term - mx).sum())
    return alpha


def kernel(decoded, transitions, raw_outputs=None, outputs=None, _backend="hw"):
    decoded = np.asarray(decoded, dtype=np.float32)
    transitions = np.asarray(transitions, dtype=np.float32)

    in_maps, sigma = _prepare_core_inputs(decoded, transitions)

    if "nc" not in _CACHE:
        _CACHE["nc"] = _build_program()
    nc = _CACHE["nc"]

    if _backend == "sim":
        from concourse.bass_interp import CoreSim
        results = []
        for c in range(NCORES):
            sim = CoreSim(nc, trace=False)
            for k, v in in_maps[c].items():
                sim.tensor(k)[:] = v
            sim.simulate()
            results.append({k: np.array(sim.tensor(k))
                            for k in ("snapd", "snapl", "vfin")})
    else:
        from concourse.bass_utils import run_bass_kernel_spmd
        res = run_bass_kernel_spmd(nc, in_maps, list(range(NCORES)))
        results = res.results

    alpha = _assemble(transitions, results, sigma, decoded)
    return np.float32(alpha)
